# revision 34
# baseline (speedup 1.0000x reference)
"""Trainium2 Bass kernel for nn_Mnn_Conv2d_Compose_without_Rho (v2).

Math (per channel c, with BN batch stats mu, v over (N,H,W)):
  m   = conv3x3(mean, w, pad=1) + b
  var = conv3x3(std^2, w^2, pad=1)
  q_c = beta*sqrt(v+eps)/gamma - mu
  z   = (m + q_c) * rk,   rk = (2*var + 2*TINY)^-1/2
  e   = erf(z)
  u_p = 0.125*S_e + 0.5        (S_e = 2x2 window sum of e)
  s_p = sqrt(1/16 - S_t/64)    (S_t = 2x2 window sum of e^2; S_t <= 4 exact)

v2 design (vs v1 = all-bf16 convs, 284us):
  - mean conv fp16 with taps 0..3 as TWO fp8-e4m3 DoubleRow matmuls (7
    streams/row-tile instead of 9). fp16 weights pre-scaled by 2^17 so the
    fp8 pairs (x*32)*(w*4096) land on the same PSUM scale; eviction descales.
    fp8 noise on a random-sign conv costs ~2.04e-2*sqrt(k/9) L2 error for k
    fp8 taps; k=4 -> 1.37e-2 vs the 2e-2 gate (KFP8TAPS env overrides).
  - var conv entirely fp8-e4m3 DoubleRow: tap pairs (0,1)(2,3)(4,5)(6,7) as 4
    DR matmuls + tap 8 single (5 streams instead of 9). fp8 is error-FREE here
    because all products are positive (coherent sum >> incoherent noise).
  - Phase A b-outer: block 0's BN stats complete after 4 chunks (~65us) so its
    AllGather hides under the rest of phase A; block 1's hides under phase
    B(b0). AllGather + local DVE reduce (Shared-addr-space out) instead of
    AllReduce; stats sumsq split in half-chunks off the eviction critical path.
  - Phase B b-outer with per-chunk phase-C work pipelined under the conv
    window, split front (z-stt, erf, e^2, col-pair adds on DVE; row-pair adds
    on GPSIMD) / back (u-final ts + DMA, w-row add) to avoid DVE FIFO
    head-of-line stalls behind GPSIMD.
  - ONE merged ACT chain (dep edges) in burst order [rk rk][erf erf]: rsqrt
    evictions stay prompt (PSUM release) and table switches are halved; tail
    sqrt regime entered once; final chunk's phase-C split in half-chunks.
Sharding: batch dim across 8 cores (4 images each); weights replicated;
BN sums/sumsq AllGather'd per cout block (1KB each).
"""
import os
import numpy as np
import ml_dtypes

import concourse.bass as bass
import concourse.bacc as bacc
import concourse.tile as tile
import concourse.mybir as mybir
from concourse import bass_utils
from concourse.tile_rust import add_dep_helper

AF = mybir.ActivationFunctionType
ALU = mybir.AluOpType
DRMODE = mybir.MatmulPerfMode.DoubleRow
F16 = np.float16
F8 = ml_dtypes.float8_e4m3
F32 = np.float32
DT16 = mybir.dt.float16
DT8 = mybir.dt.float8e4
DT32 = mybir.dt.float32

NCORES = 8
B_GLOBAL = 32
BC = B_GLOBAL // NCORES          # images per core
CIN = 128
COUT = 256
NB = COUT // 128                 # cout blocks
H = W = 56
HP = WP = 58                     # padded
NPIX = H * W                     # 3136
NHW = B_GLOBAL * NPIX            # global batch-norm count
TINY = 1e-12
BN_EPS = 1e-5
RT = 7                           # row tiles of 8 rows each
RROWS = 8
RN = RROWS * W                   # 448 pixels per row tile

TAPS = [(t // 3, t % 3) for t in range(9)]
VPAIRS = [(0, 1), (2, 3), (4, 5), (6, 7)]   # var-conv DR tap pairs

FP8_TAPS = int(os.environ.get("KFP8TAPS", "4"))   # mean-conv fp8 taps (0/2/4)
SX8, SW8 = 32.0, 4096.0          # mean fp8 scales
WSC = SX8 * SW8 if FP8_TAPS else 1.0   # fp16 mean-weight upscale
SS8, SW28 = 128.0, float(2 ** 17)      # var conv fp8 scales

LAST_RESULTS = None              # populated by kernel() for test harness


def _act_raw(nc, out, in_, func, bias_ap, scale=1.0, accum_out=None):
    """Raw InstActivation emit (used for Rsqrt, which activation() refuses)."""
    eng = nc.scalar
    ins = [eng.lower_ap(in_),
           eng.lower_ap(bias_ap),
           mybir.ImmediateValue(dtype=mybir.dt.float32, value=float(scale)),
           mybir.ImmediateValue(dtype=mybir.dt.float32, value=0.0)]
    outs = [eng.lower_ap(out)]
    if accum_out is not None:
        outs.append(eng.lower_ap(accum_out))
    return eng.add_instruction(
        mybir.InstActivation(
            name=nc.get_next_instruction_name(),
            func=func, ins=ins, outs=outs))


def _pair_rhs(x_t, r, ta, tb):
    """AP [128, 2, 8, 56]: k-tile 0 = tap ta, k-tile 1 = tap tb."""
    kya, kxa = TAPS[ta]
    kyb, kxb = TAPS[tb]
    base = x_t[:, RROWS * r + kya: RROWS * r + kya + RROWS, kxa: kxa + W]
    u = base.unsqueeze(1)
    delta = (kyb - kya) * WP + (kxb - kxa)
    apl = u.ap
    apl[1] = (delta, 2)
    u.ap = apl
    return u


def _build():
    # KPHASES bisect knob: A (mean conv only), AC (+collective),
    # AB (+var conv), full (everything)
    PH = os.environ.get("KPHASES", "full")
    do_coll = PH in ("AC", "AB", "full")
    do_B = PH in ("AB", "full")
    do_C = PH == "full"

    nc = bacc.Bacc("TRN2", target_bir_lowering=False, debug=False,
                   enable_asserts=True, num_devices=NCORES)

    xm16 = nc.dram_tensor("xm16", [BC, CIN, HP, WP], DT16, kind="ExternalInput")
    if FP8_TAPS:
        xm8 = nc.dram_tensor("xm8", [BC, CIN, HP, WP], DT8, kind="ExternalInput")
        w8p = nc.dram_tensor("w8p", [CIN, FP8_TAPS, COUT], DT8,
                             kind="ExternalInput")
    xs2 = nc.dram_tensor("xs2", [BC, CIN, HP, WP], DT8, kind="ExternalInput")
    wt = nc.dram_tensor("wt", [CIN, 9, COUT], DT16, kind="ExternalInput")
    w2t = nc.dram_tensor("w2t", [CIN, 9, COUT], DT8, kind="ExternalInput")
    cb = nc.dram_tensor("cb", [128, NB], DT32, kind="ExternalInput")
    bg = nc.dram_tensor("bg", [128, NB], DT32, kind="ExternalInput")
    out_u = nc.dram_tensor("out_u", [BC, COUT, 784], DT16, kind="ExternalOutput")
    out_s = nc.dram_tensor("out_s", [BC, COUT, 784], DT16, kind="ExternalOutput")

    ev_stream = []    # PSUM-critical ACT evictions (identity + rsqrt), PE order
    erf_stream = []   # q-rsqrts, erfs, tail sqrts, in order
    cross_deps = []   # (later, earlier) one-way edges erf<-rk

    with tile.TileContext(nc) as tc:
        with (
            tc.tile_pool(name="xm16p", bufs=3) as xm16_pool,
            tc.tile_pool(name="xm8p", bufs=3) as xm8_pool,
            tc.tile_pool(name="xs2p", bufs=3) as xs2_pool,
            tc.tile_pool(name="wp", bufs=1) as w_pool,
            tc.tile_pool(name="big", bufs=1) as big_pool,
            tc.tile_pool(name="rkp", bufs=4) as rk_pool,
            tc.tile_pool(name="scr", bufs=1) as scr_pool,
            tc.tile_pool(name="ep", bufs=2) as e_pool,
            tc.tile_pool(name="tp", bufs=2) as t_pool,
            tc.tile_pool(name="pxp", bufs=3) as px_pool,
            tc.tile_pool(name="sep", bufs=3) as se_pool,
            tc.tile_pool(name="spp", bufs=2) as sp_pool,
            tc.tile_pool(name="psA", bufs=1, space="PSUM") as psA_pool,
            tc.tile_pool(name="psB", bufs=1, space="PSUM") as psB_pool,
            tc.tile_pool(name="dram", bufs=1, space="DRAM") as dram_pool,
        ):
            # ---- persistent tiles ----
            w_sb = w_pool.tile([CIN, 9, COUT], DT16, tag="w")
            w2_sb = w_pool.tile([CIN, 9, COUT], DT8, tag="w2")
            cb_sb = w_pool.tile([128, NB], DT32, tag="cb")
            bg_sb = w_pool.tile([128, NB], DT32, tag="bg")
            # head-critical DMAs first: rows 0..20 of image 0 + w block 0
            x0_t = xm16_pool.tile([CIN, HP, WP], DT16, tag="xm16", name="x0")
            nc.sync.dma_start(x0_t[:, 0:20, :], xm16.ap()[0, :, 0:20, :])
            nc.sync.dma_start(w_sb[:, :, 0:128], wt.ap()[:, :, 0:128])
            nc.sync.dma_start(x0_t[:, 20:HP, :], xm16.ap()[0, :, 20:HP, :])
            nc.sync.dma_start(w_sb[:, :, 128:COUT], wt.ap()[:, :, 128:COUT])
            if FP8_TAPS:
                w8p_sb = w_pool.tile([CIN, FP8_TAPS, COUT], DT8, tag="w8p")
                x0_8t = xm8_pool.tile([CIN, HP, WP], DT8, tag="xm8", name="x08")
                nc.sync.dma_start(x0_8t[:], xm8.ap()[0])
                nc.sync.dma_start(w8p_sb[:], w8p.ap())
            nc.sync.dma_start(w2_sb[:], w2t.ap())
            nc.sync.dma_start(cb_sb[:], cb.ap())
            nc.sync.dma_start(bg_sb[:], bg.ap())

            zero_b = w_pool.tile([128, 1], DT32, tag="zb")
            nc.vector.memset(zero_b[:], 0.0)
            tiny2_b = w_pool.tile([128, 1], DT32, tag="tb")
            nc.vector.memset(tiny2_b[:], 2.0 * TINY)
            half_b = w_pool.tile([128, 1], DT32, tag="hb")
            nc.vector.memset(half_b[:], 0.5)
            sixt_b = w_pool.tile([128, 1], DT32, tag="xb")
            nc.vector.memset(sixt_b[:], 1.0 / 16.0)

            m_sb = big_pool.tile([128, NB, BC, NPIX], DT16, tag="m")
            dst_sb = big_pool.tile([128, BC, NB, 784], DT16, tag="dst")

            sum_sc = scr_pool.tile([128, NB, 2 * BC], DT32, tag="sums")
            ssq_sc = scr_pool.tile([128, NB, 2 * BC], DT32, tag="ssq")
            stats = [scr_pool.tile([128, 2], DT32, tag=f"stats{b}",
                                   name=f"stats{b}") for b in range(NB)]
            gstats = [scr_pool.tile([128, NCORES, 2], DT32, tag=f"gstats{b}",
                                    name=f"gstats{b}") for b in range(NB)]
            gsum = [scr_pool.tile([128, 2], DT32, tag=f"gsum{b}",
                                  name=f"gsum{b}") for b in range(NB)]
            q_t = scr_pool.tile([128, NB], DT32, tag="q")

            # ---------------- Phase A: mean conv (b outer!) ----------------
            # b-outer means block 0's BN stats are complete after only 4
            # chunks -> its AllReduce hides under the rest of phase A even
            # at ~40us collective latency. Images are re-DMA'd per block.
            for b in range(NB):
                for n in range(BC):
                    if b == 0 and n == 0:
                        x_t = x0_t
                        x8_t = x0_8t if FP8_TAPS else None
                    else:
                        x_t = xm16_pool.tile([CIN, HP, WP], DT16, tag="xm16")
                        nc.sync.dma_start(x_t[:], xm16.ap()[n])
                        if FP8_TAPS:
                            x8_t = xm8_pool.tile([CIN, HP, WP], DT8, tag="xm8")
                            nc.sync.dma_start(x8_t[:], xm8.ap()[n])
                    bsl = slice(128 * b, 128 * (b + 1))
                    psA = psA_pool.tile([128, 4, 512], DT32, tag="psA")
                    psB = psB_pool.tile([128, 3, 512], DT32, tag="psB")

                    def evict_m(ps_ap, half, n=n, b=b):
                        npx = ps_ap.shape[1] * RN
                        off = 0 if half == 0 else 4 * RN
                        ev = nc.scalar.activation(
                            m_sb[:, b, n, off: off + npx], ps_ap,
                            AF.Identity, bias=cb_sb[:, b: b + 1],
                            scale=1.0 / WSC,
                            accum_out=sum_sc[:, b, 2 * n + half:
                                             2 * n + half + 1])
                        ev_stream.append(ev)
                        return ev

                    def emit_ssq(half, n=n, b=b):
                        # sumsq of m half via DVE stt m*1*m with accum; the
                        # elementwise out is dumped into the e_pool ring.
                        # Split in halves so the last piece off the final
                        # eviction is short (stats critical path).
                        lo, hi = (0, 4 * RN) if half == 0 else (4 * RN, NPIX)
                        dump = e_pool.tile([128, NPIX], DT16, tag="e32",
                                           name="dump")
                        nc.vector.scalar_tensor_tensor(
                            dump[:, 0:hi - lo], m_sb[:, b, n, lo:hi], 1.0,
                            m_sb[:, b, n, lo:hi], op0=ALU.mult, op1=ALU.mult,
                            accum_out=ssq_sc[:, b, 2 * n + half:
                                             2 * n + half + 1])

                    for r in range(RT):
                        ps = psA[:, r, 0:RN] if r < 4 else psB[:, r - 4, 0:RN]
                        # fp16 taps first (head: x0/w arrive before xm8/w8p)
                        for t9 in range(FP8_TAPS, 9):
                            ky, kx = TAPS[t9]
                            rhs = x_t[:, RROWS * r + ky: RROWS * r + ky + RROWS,
                                      kx: kx + W]
                            nc.tensor.matmul(ps, w_sb[:, t9, bsl], rhs,
                                             start=(t9 == FP8_TAPS),
                                             stop=(t9 == 8 and not FP8_TAPS))
                        for p8 in range(FP8_TAPS // 2):
                            nc.tensor.matmul(
                                ps, w8p_sb[:, 2 * p8:2 * p8 + 2, bsl],
                                _pair_rhs(x8_t, r, 2 * p8, 2 * p8 + 1),
                                start=False, stop=(p8 == FP8_TAPS // 2 - 1),
                                perf_mode=DRMODE)
                        if r == 3:
                            evict_m(psA[:, 0:4, 0:RN], 0)
                            emit_ssq(0)
                    evict_m(psB[:, 0:3, 0:RN], 1)
                    emit_ssq(1)

                    # per-block stats -> AllReduce immediately after block b's
                    # last chunk; DMAs ride the DVE's own queue so they are
                    # not stuck behind bulk input loads on the sync queue
                    if n == BC - 1 and do_coll:
                        nc.vector.tensor_reduce(
                            stats[b][:, 0:1], sum_sc[:, b, :],
                            axis=mybir.AxisListType.X, op=ALU.add)
                        nc.vector.tensor_reduce(
                            stats[b][:, 1:2], ssq_sc[:, b, :],
                            axis=mybir.AxisListType.X, op=ALU.add)
                        cc_in = dram_pool.tile([128, 2], DT32, tag=f"cci{b}")
                        cc_out = dram_pool.tile([NCORES, 128, 2], DT32,
                                                tag=f"cco{b}",
                                                addr_space="Shared")
                        nc.sync.dma_start(cc_in[:], stats[b][:])
                        nc.gpsimd.collective_compute(
                            "AllGather", ALU.bypass,
                            replica_groups=[list(range(NCORES))],
                            ins=[cc_in.opt()], outs=[cc_out.opt()])
                        nc.sync.dma_start(
                            gstats[b][:],
                            cc_out[:].rearrange("c p j -> p c j"))

            # per-block q scratch (emission of the ops happens inside the
            # phase-B loop so the DVE FIFO order matches data readiness)
            mu_t = scr_pool.tile([128, NB], DT32, tag="mu")
            ex2_t = scr_pool.tile([128, NB], DT32, tag="ex2")
            v_t = scr_pool.tile([128, NB], DT32, tag="v")
            rsq_t = scr_pool.tile([128, NB], DT32, tag="rsq")
            sv_t = scr_pool.tile([128, NB], DT32, tag="sv")

            def emit_q_chain(b):
                """q_b = beta/gamma*sqrt(v+eps) - mu; returns the ACT rsqrt."""
                bb = slice(b, b + 1)
                nc.vector.tensor_reduce(
                    gsum[b][:], gstats[b][:].rearrange("p c j -> p j c"),
                    axis=mybir.AxisListType.X, op=ALU.add)
                nc.vector.tensor_scalar_mul(mu_t[:, bb], gsum[b][:, 0:1],
                                            1.0 / NHW)
                nc.vector.tensor_scalar_mul(ex2_t[:, bb], gsum[b][:, 1:2],
                                            1.0 / NHW)
                nc.vector.tensor_mul(v_t[:, bb], mu_t[:, bb], mu_t[:, bb])
                nc.vector.tensor_sub(v_t[:, bb], ex2_t[:, bb], v_t[:, bb])
                nc.vector.tensor_scalar_add(v_t[:, bb], v_t[:, bb], BN_EPS)
                qrs = _act_raw(nc, rsq_t[:, bb], v_t[:, bb], AF.Rsqrt,
                               zero_b[:], scale=1.0)
                nc.vector.tensor_mul(sv_t[:, bb], v_t[:, bb], rsq_t[:, bb])
                nc.vector.tensor_mul(sv_t[:, bb], sv_t[:, bb], bg_sb[:, bb])
                nc.vector.tensor_sub(q_t[:, bb], sv_t[:, bb], mu_t[:, bb])
                return qrs

            # ---------------- Phase B: var conv (b outer) + phase C --------
            def emit_cwork_front(b, n, rk_t, r0=0, r1=H):
                """z -> erf -> e^2 -> col-pair + row-pair adds (front half)."""
                nr = r1 - r0
                po, pn = r0 * W, nr * W
                hf = nr * 28
                qo, qn = (r0 // 2) * 28, (nr // 2) * 28
                m_ap = m_sb[:, b, n, po:po + pn]
                # z = (m + q_b) * rk, in place over m (fp16, DVE)
                nc.vector.scalar_tensor_tensor(
                    m_ap, m_ap, q_t[:, b: b + 1], rk_t[:, po:po + pn],
                    op0=ALU.add, op1=ALU.mult)
                e32 = e_pool.tile([128, NPIX], DT16, tag="e32")
                erf_i = nc.scalar.activation(e32[:, 0:pn], m_ap, AF.Erf,
                                             bias=zero_b[:], scale=1.0)
                t32 = t_pool.tile([128, NPIX], DT16, tag="t32")
                nc.vector.tensor_mul(t32[:, 0:pn], e32[:, 0:pn], e32[:, 0:pn])

                e3 = e32[:, 0:pn].rearrange("p (r c2 cp) -> p r c2 cp",
                                            c2=28, cp=2)
                ex_t = px_pool.tile([128, H * 28], DT16, tag="ex")
                exv = ex_t[:, 0:hf].rearrange("p (r c) -> p r c", c=28)
                nc.vector.tensor_add(exv, e3[:, :, :, 0], e3[:, :, :, 1])
                ex4 = ex_t[:, 0:hf].rearrange("p (r2 rp c2) -> p r2 rp c2",
                                              rp=2, c2=28)
                se_t = se_pool.tile([128, 784], DT16, tag="se")
                nc.gpsimd.tensor_add(
                    se_t[:, qo:qo + qn].rearrange("p (a b) -> p a b", b=28),
                    ex4[:, :, 0, :], ex4[:, :, 1, :])
                t3 = t32[:, 0:pn].rearrange("p (r c2 cp) -> p r c2 cp",
                                            c2=28, cp=2)
                wx_t = px_pool.tile([128, H * 28], DT16, tag="wx")
                wxv = wx_t[:, 0:hf].rearrange("p (r c) -> p r c", c=28)
                nc.gpsimd.tensor_add(wxv, t3[:, :, :, 0], t3[:, :, :, 1])
                return erf_i, (b, n, r0, r1, se_t, wx_t)

            def emit_cwork_back(ctx):
                """u final scale + DMA, w row-pair -> dst (back half)."""
                b, n, r0, r1, se_t, wx_t = ctx
                bsl = slice(128 * b, 128 * (b + 1))
                nr = r1 - r0
                hf = nr * 28
                qo, qn = (r0 // 2) * 28, (nr // 2) * 28
                uo16 = se_pool.tile([128, 784], DT16, tag="uo16")
                nc.vector.tensor_scalar(uo16[:, 0:qn], se_t[:, qo:qo + qn],
                                        0.125, 0.5, op0=ALU.mult, op1=ALU.add)
                nc.sync.dma_start(out_u.ap()[n, bsl, qo:qo + qn],
                                  uo16[:, 0:qn])
                wx4 = wx_t[:, 0:hf].rearrange("p (r2 rp c2) -> p r2 rp c2",
                                              rp=2, c2=28)
                nc.vector.tensor_add(
                    dst_sb[:, n, b, qo:qo + qn].rearrange(
                        "p (a b) -> p a b", b=28),
                    wx4[:, :, 0, :], wx4[:, :, 1, :])

            # merged ACT chain in burst order: [rk rk][erf erf] with 2-chunk
            # bursts halves the table loads vs per-chunk alternation, and
            # rks stay prompt (PSUM release) because only 2 erfs ever sit
            # between consecutive rk groups.
            pending = []             # conv'd chunks awaiting cwork
            for b in range(NB) if do_B else []:
                bsl = slice(128 * b, 128 * (b + 1))
                for n in range(BC):
                    x_t = xs2_pool.tile([CIN, HP, WP], DT8, tag="xs2")
                    nc.sync.dma_start(x_t[:], xs2.ap()[n])
                    psA = psA_pool.tile([128, 4, 512], DT32, tag="psA")
                    psB = psB_pool.tile([128, 3, 512], DT32, tag="psB")
                    rk_t = rk_pool.tile([128, NPIX], DT16, tag="rk")

                    for r in range(RT):
                        ps = psA[:, r, 0:RN] if r < 4 else psB[:, r - 4, 0:RN]
                        for p, (ta, tb) in enumerate(VPAIRS):
                            nc.tensor.matmul(ps, w2_sb[:, ta:ta + 2, bsl],
                                             _pair_rhs(x_t, r, ta, tb),
                                             start=(p == 0), stop=False,
                                             perf_mode=DRMODE)
                        ky, kx = TAPS[8]
                        rhs = x_t[:, RROWS * r + ky: RROWS * r + ky + RROWS,
                                  kx: kx + W]
                        nc.tensor.matmul(ps, w2_sb[:, 8, bsl], rhs,
                                         start=False, stop=True)
                        if r == 3:
                            ev = _act_raw(nc, rk_t[:, 0:4 * RN],
                                          psA[:, 0:4, 0:RN], AF.Rsqrt,
                                          tiny2_b[:], scale=2.0 / (SS8 * SW28))
                            ev_stream.append(ev)
                    ev = _act_raw(nc, rk_t[:, 4 * RN:7 * RN],
                                  psB[:, 0:3, 0:RN], AF.Rsqrt,
                                  tiny2_b[:], scale=2.0 / (SS8 * SW28))
                    ev_stream.append(ev)
                    pending.append((b, n, rk_t))
                    # burst: once 3 chunks are pending, cwork the oldest 2
                    if do_C and len(pending) >= 3:
                        ctxs = []
                        for bb_, nn_, rkt_ in pending[:-1]:
                            erf_i, ctx = emit_cwork_front(bb_, nn_, rkt_)
                            ev_stream.append(erf_i)
                            ctxs.append(ctx)
                        for ctx in ctxs:
                            emit_cwork_back(ctx)
                        pending = pending[-1:]
                    if n == 0 and do_coll:
                        # q-chain after the burst pop so the DVE FIFO order
                        # matches data readiness; its tiny rsqrt joins the
                        # NEXT chunk's rsqrt burst in the chain
                        ev_stream.append(emit_q_chain(b))
            if do_C and pending:
                # drain: remaining chunks; final one in half-chunks so the
                # serial z->erf->pool chain after the last matmul is short
                ctxs = []
                for bb_, nn_, rkt_ in pending[:-1]:
                    erf_i, ctx = emit_cwork_front(bb_, nn_, rkt_)
                    ev_stream.append(erf_i)
                    ctxs.append(ctx)
                bb_, nn_, rkt_ = pending[-1]
                for rr in ((0, 32), (32, H)):
                    erf_i, ctx = emit_cwork_front(bb_, nn_, rkt_, rr[0], rr[1])
                    ev_stream.append(erf_i)
                    ctxs.append(ctx)
                for ctx in ctxs:
                    emit_cwork_back(ctx)

            # ---------------- tail: s_p = sqrt(1/16 - S_t/64) --------------
            for n in range(BC) if do_C else []:
                sp_t = sp_pool.tile([128, NB, 784], DT16, tag="sp16")
                sq_i = nc.scalar.activation(
                    sp_t[:].rearrange("p a b -> p (a b)"),
                    dst_sb[:, n, :, :].rearrange("p a b -> p (a b)"),
                    AF.Sqrt, bias=sixt_b[:], scale=-1.0 / 64.0)
                ev_stream.append(sq_i)
                for b in range(NB):
                    nc.sync.dma_start(out_s.ap()[n, 128 * b: 128 * (b + 1), :],
                                      sp_t[:, b, :])

            # ---- single merged ACT chain pins engine order + table sets ---
            for a, bp in zip(ev_stream[1:], ev_stream[:-1]):
                add_dep_helper(a.ins, bp.ins, sync=False,
                               reason="act chain order")

    nc.compile()
    return nc


_CACHE = {}


def _get_nc():
    key = (os.environ.get("KPHASES", "full"), FP8_TAPS)
    if key not in _CACHE:
        _CACHE[key] = _build()
    return _CACHE[key]


def kernel(mean, std, conv_w, conv_b, bn_gamma, bn_beta):
    global LAST_RESULTS
    mean = np.asarray(mean)
    std = np.asarray(std)
    conv_w = np.asarray(conv_w)
    conv_b = np.asarray(conv_b)
    bn_gamma = np.asarray(bn_gamma)
    bn_beta = np.asarray(bn_beta)

    # ---- host-side prep (layout + dtype/scale casts only) ----
    xm16 = np.zeros((B_GLOBAL, CIN, HP, WP), F16)
    xm16[:, :, 1:57, 1:57] = mean.astype(F16)
    xs2 = np.zeros((B_GLOBAL, CIN, HP, WP), F8)
    xs2[:, :, 1:57, 1:57] = ((std.astype(F32) ** 2) * SS8).astype(F8)
    wtr = np.ascontiguousarray(
        conv_w.astype(F32).transpose(1, 2, 3, 0).reshape(CIN, 9, COUT))
    wt = (wtr * WSC).astype(F16)
    w2t = ((wtr ** 2) * SW28).astype(F8)
    cbh = np.ascontiguousarray(conv_b.astype(F32).reshape(NB, 128).T)
    bgh = np.ascontiguousarray(
        (bn_beta.astype(F32) / bn_gamma.astype(F32)).reshape(NB, 128).T)

    in_common = dict(wt=wt, w2t=w2t, cb=cbh, bg=bgh)
    if FP8_TAPS:
        xm8 = np.zeros((B_GLOBAL, CIN, HP, WP), F8)
        xm8[:, :, 1:57, 1:57] = (mean.astype(F32) * SX8).astype(F8)
        in_common["w8p"] = np.ascontiguousarray(
            (wtr[:, :FP8_TAPS, :] * SW8)).astype(F8)

    in_maps = []
    for c in range(NCORES):
        sl = slice(BC * c, BC * (c + 1))
        m = dict(xm16=np.ascontiguousarray(xm16[sl]),
                 xs2=np.ascontiguousarray(xs2[sl]), **in_common)
        if FP8_TAPS:
            m["xm8"] = np.ascontiguousarray(xm8[sl])
        in_maps.append(m)

    nc = _get_nc()
    res = bass_utils.run_bass_kernel_spmd(
        nc, in_maps, core_ids=list(range(NCORES)),
        trace=bool(os.environ.get("KBENCH_TRACE")))
    LAST_RESULTS = res

    u = np.concatenate([res.results[c]["out_u"].reshape(BC, COUT, 28, 28)
                        for c in range(NCORES)], axis=0).astype(F32)
    s = np.concatenate([res.results[c]["out_s"].reshape(BC, COUT, 28, 28)
                        for c in range(NCORES)], axis=0).astype(F32)
    return (u, s)
